# revision 3
# baseline (speedup 1.0000x reference)
"""Trainium2 Bass kernel for BroadcastObstaclesToLanes (embedding lookup).

out[m, :] = obs_pos[same_obs_mask[m, 0], :]   m in [0, 16777216)

Sharding: M (lanes) split across 8 NeuronCores; the obs_pos table is
replicated so every core's gather is fully local.

Per core (2,097,152 lanes): lanes are gathered 128 at a time with SWDGE
indirect DMA in partition form (dest [128, 2] f32; one 8B descriptor per
partition, descriptor p reads obs_pos[idx[p]] from HBM). 128 consecutive
gathers share a [128, 256] f32 staging tile (gather i writes columns
2i:2i+2), which is then copied out with one efficient 1KB-per-partition DMA.

Lane layout: lane (chunk, i, p) = chunk*16384 + i*128 + p lives at
staging/dram position [chunk][p][2i:2i+2]; the host view/transposes the
index input and the output accordingly (pure layout, no value compute).
"""

import os

import numpy as np

N_OBS = 1048576
M_LANES = 16777216
NCORES = 8
MS = M_LANES // NCORES  # 2,097,152 lanes per core
P = 128
NI = MS // P  # 16384 gather instructions per core
CHUNK = 128  # gathers per staging flush
NCH = NI // CHUNK  # 128 chunks

_cached_nc = None


def _build():
    global _cached_nc
    if _cached_nc is not None:
        return _cached_nc

    import concourse.bacc as bacc
    import concourse.tile as tile
    from concourse import mybir
    from concourse.bass import IndirectOffsetOnAxis

    nc = bacc.Bacc(
        "TRN2", target_bir_lowering=False, debug=False, num_devices=NCORES
    )
    tbl = nc.dram_tensor(
        "tbl", [N_OBS, 2], mybir.dt.float32, kind="ExternalInput"
    ).ap()
    # idxs[p][n] = lane index for lane n*128 + p (column-major lane blocks)
    idxs_d = nc.dram_tensor(
        "idxs", [P, NI], mybir.dt.int32, kind="ExternalInput"
    ).ap()
    # out[ch][p][2i+d] = gathered pair for lane ch*16384 + i*128 + p
    out = nc.dram_tensor(
        "out", [NCH, P, CHUNK * 2], mybir.dt.float32, kind="ExternalOutput"
    ).ap()

    with tile.TileContext(nc) as tc:
        with tc.tile_pool(name="io", bufs=1) as pool, tc.tile_pool(
            name="st", bufs=4
        ) as stpool:
            idxmega = pool.tile([P, NI], mybir.dt.int32, tag="idx")
            nc.scalar.dma_start(idxmega[:], idxs_d[:])
            for ch in range(NCH):
                stag = stpool.tile([P, CHUNK * 2], mybir.dt.float32, tag="stag")
                for i in range(CHUNK):
                    n = ch * CHUNK + i
                    nc.gpsimd.indirect_dma_start(
                        out=stag[:, 2 * i : 2 * i + 2],
                        out_offset=None,
                        in_=tbl[:],
                        in_offset=IndirectOffsetOnAxis(
                            ap=idxmega[:, n : n + 1], axis=0
                        ),
                    )
                nc.sync.dma_start(out[ch], stag[:])

    nc.compile()
    _cached_nc = nc
    return nc


def kernel(obs_pos, same_obs_mask):
    from concourse.bass_utils import run_bass_kernel_spmd

    nc = _build()

    tbl = np.ascontiguousarray(np.asarray(obs_pos, dtype=np.float32))
    idx32 = np.asarray(same_obs_mask).reshape(-1).astype(np.int32)

    in_maps = []
    for c in range(NCORES):
        lanes = idx32[c * MS : (c + 1) * MS]
        # [MS] -> [P, NI]: idxs[p][n] = lanes[n*128 + p]
        in_maps.append(
            {
                "tbl": tbl,
                "idxs": np.ascontiguousarray(lanes.reshape(NI, P).T),
            }
        )
    res = run_bass_kernel_spmd(nc, in_maps, core_ids=list(range(NCORES)))
    outs = []
    for r in res.results:
        o = r["out"].reshape(NCH, P, CHUNK, 2)  # [ch][p][i][d]
        outs.append(o.transpose(0, 2, 1, 3).reshape(MS, 2))  # lane-major
    return np.ascontiguousarray(np.concatenate(outs, axis=0))
